# revision 10
# baseline (speedup 1.0000x reference)
import sys, os
import numpy as np

for p in ("/root/.axon_site/_ro/trn_rl_repo", "/opt/trn_rl_repo"):
    if os.path.isdir(p) and p not in sys.path:
        sys.path.insert(0, p)

import concourse.bass as bass
import concourse.mybir as mybir
import concourse.tile as tile
import concourse.bacc as bacc
from concourse.bass_utils import run_bass_kernel_spmd

F32 = mybir.dt.float32
NE, NU, NA, NSV, NPV = 512, 256, 16, 256, 64
SLAB = NE // 8


def _apx(ap, dims):
    # ap is a [P, 1]-style slice; set its free dims to `dims` (partition dim kept)
    if len(dims) == 1:
        ap.ap[-1] = list(dims[0])
        return ap
    assert len(dims) == 2
    r = ap.rearrange("a (b c) -> a b c", b=1, c=1)
    r.ap[1] = list(dims[0])
    r.ap[2] = list(dims[1])
    return r


def _dap(t, dims, off=0):
    # DRAM access pattern with explicit [step, count] dims and element offset
    s = t.flatten()
    base = s.offset
    while s.ndim < len(dims):
        s = s.unsqueeze(0)
    for i, d in enumerate(dims):
        s.ap[i] = list(d)
    s.offset = base + off
    return s


def build():
    nc = bacc.Bacc("TRN2", target_bir_lowering=False, debug=False, num_devices=8)
    D = {}
    def din(name, shape):
        D[name] = nc.dram_tensor(name, shape, F32, kind="ExternalInput").ap()
        return D[name]

    rT = din("rT", [3, NE]); rsT = din("rsT", [3, SLAB])
    aT3 = din("aT3", [3, NA]); aoff = din("aoff", [48, 1]); na2 = din("na2", [NA, 1])
    V0p = din("V0p", [200, NSV]); V0b = din("V0b", [NSV, 1])
    for l in range(3):
        din(f"Vr{l}", [896, NSV]); din(f"Vrb{l}", [NSV, 1])
        din(f"Wr{l}", [NPV, NPV]); din(f"Wrb{l}", [NPV, 1])
    W0abc = din("W0abc", [3, NPV]); W0d = din("W0d", [1, NPV]); W0b = din("W0b", [NPV, 1])
    din("Vhu", [896, NSV]); din("Vhub", [NSV, 1])
    din("Vhd", [896, NSV]); din("Vhdb", [NSV, 1])
    din("wu", [NSV, NU]); din("wub", [NU, 1])
    din("wd", [NSV, NU]); din("wdb", [NU, 1])
    diag3 = din("diag3", [4, 128, SLAB])
    sel3 = din("sel3", [3, 48])
    orbu = nc.dram_tensor("orbu", [NU, NU], F32, kind="ExternalOutput").ap()
    orbd = nc.dram_tensor("orbd", [NU, NU], F32, kind="ExternalOutput").ap()

    AG8 = [list(range(8))]
    AF = mybir.ActivationFunctionType
    AL = mybir.AluOpType
    X = mybir.AxisListType.X

    with tile.TileContext(nc) as tc:
        with (
            tc.tile_pool(name="big", bufs=1) as big,
            tc.tile_pool(name="sb", bufs=1) as sb,
            tc.tile_pool(name="wp", bufs=1) as wp,
            tc.tile_pool(name="lp", bufs=4) as lp,
            tc.tile_pool(name="ps", bufs=3, space="PSUM") as ps,
            tc.tile_pool(name="ps1", bufs=2, space="PSUM") as ps1,
            tc.tile_pool(name="dr", bufs=1, space="DRAM") as dr,
        ):
            ones3 = sb.tile([3, 1], F32); nc.gpsimd.memset(ones3[:], 1.0)
            ones16 = sb.tile([NA, 1], F32); nc.gpsimd.memset(ones16[:], 1.0)
            ones128 = sb.tile([128, 1], F32); nc.gpsimd.memset(ones128[:], 1.0)
            ones1r = sb.tile([1, 128], F32); nc.gpsimd.memset(ones1r[:], 1.0)
            idn = sb.tile([128, 128], F32)
            nc.gpsimd.memset(idn[:], 1.0)
            nc.gpsimd.affine_select(out=idn[:], in_=idn[:], pattern=[[-1, 128]],
                                    compare_op=AL.is_equal, fill=0.0, base=0, channel_multiplier=1)

            rT_s = sb.tile([3, NE], F32); nc.sync.dma_start(rT_s[:], rT[:])
            rsT_s = sb.tile([3, SLAB], F32); nc.sync.dma_start(rsT_s[:], rsT[:])
            aT3_s = sb.tile([3, NA], F32); nc.sync.dma_start(aT3_s[:], aT3[:])
            aoff_s = sb.tile([48, 1], F32); nc.sync.dma_start(aoff_s[:], aoff[:])
            na2_s = sb.tile([NA, 1], F32); nc.sync.dma_start(na2_s[:], na2[:])
            sel3_s = sb.tile([3, 48], F32); nc.sync.dma_start(sel3_s[:], sel3[:])
            w0abc_s = sb.tile([3, NPV], F32); nc.sync.dma_start(w0abc_s[:], W0abc[:])
            w0d4 = sb.tile([128, NPV], F32)
            for q in range(4):
                nc.sync.dma_start(w0d4[32 * q:32 * q + 1, :], W0d[:])
            b0c = sb.tile([128, 1], F32)
            nc.sync.dma_start(b0c[0:64, :], W0b[:]); nc.sync.dma_start(b0c[64:128, :], W0b[:])
            diag3_s = big.tile([128, 4 * SLAB], F32)
            for t in range(4):
                nc.sync.dma_start(diag3_s[:, t * SLAB:(t + 1) * SLAB], diag3[t, :, :])

            # geometry
            sq3 = sb.tile([3, NE], F32)
            nc.scalar.square(sq3[:], rT_s[:])
            n2_ps = ps1.tile([1, NE], F32, tag="g")
            nc.tensor.matmul(n2_ps[:], ones3[:], sq3[:], start=True, stop=True)
            n2row = sb.tile([1, NE], F32); nc.scalar.copy(n2row[:], n2_ps[:])
            sqs = sb.tile([3, SLAB], F32)
            nc.scalar.square(sqs[:], rsT_s[:])
            n2s_ps = ps1.tile([1, SLAB], F32, tag="g")
            nc.tensor.matmul(n2s_ps[:], ones3[:], sqs[:], start=True, stop=True)
            n2s = sb.tile([1, SLAB], F32); nc.scalar.copy(n2s[:], n2s_ps[:])
            n2rep_ps = ps1.tile([128, SLAB], F32, tag="g")
            nc.tensor.matmul(n2rep_ps[:], ones1r[:], n2s[:], start=True, stop=True)
            n2rep = sb.tile([128, SLAB], F32); nc.scalar.copy(n2rep[:], n2rep_ps[:])

            rrlen_t = big.tile([128, 4 * SLAB], F32)
            for t in range(4):
                g_ps = ps1.tile([128, SLAB], F32, tag="g")
                nc.tensor.matmul(g_ps[:], rT_s[:, t * 128:(t + 1) * 128], rsT_s[:], start=True, stop=True)
                n2c_ps = ps1.tile([128, 1], F32, tag="g")
                nc.tensor.transpose(n2c_ps[:], n2row[:, t * 128:(t + 1) * 128], idn[0:1, 0:1])
                n2c = lp.tile([128, 1], F32, tag="n2c"); nc.scalar.copy(n2c[:], n2c_ps[:])
                sl = rrlen_t[:, t * SLAB:(t + 1) * SLAB]
                nc.vector.scalar_tensor_tensor(out=sl, in0=g_ps[:], scalar=-2.0, in1=n2rep[:],
                                               op0=AL.mult, op1=AL.add)
                nc.vector.tensor_scalar_add(out=sl, in0=sl, scalar1=n2c[:, 0:1])
                nc.vector.tensor_tensor(out=sl, in0=sl, in1=diag3_s[:, t * SLAB:(t + 1) * SLAB], op=AL.add)
                nc.scalar.sqrt(sl, sl)
            # rrlenT[jl, i] then rrflat4[32q, u*512 + i] = rrlen[(i, jl=q+4u)]
            rrlenT = big.tile([SLAB, NE], F32)
            for t in range(4):
                tp_ps = ps1.tile([SLAB, 128], F32, tag="g")
                nc.tensor.transpose(tp_ps[:], rrlen_t[:, t * SLAB:(t + 1) * SLAB], idn[:])
                nc.scalar.copy(rrlenT[:, t * 128:(t + 1) * 128], tp_ps[:])
            rrflat4 = big.tile([128, (SLAB // 4) * NE], F32)
            for q in range(4):
                dst = _apx(rrflat4[32 * q:32 * q + 1, 0:1], [[NE, SLAB // 4], [1, NE]])
                nc.sync.dma_start(dst, rrlenT[16 * q:16 * (q + 1), :])

            # qT / qs
            qT = sb.tile([128, NE], F32)
            qs = sb.tile([128, SLAB], F32)
            for h in (0, 64):
                q_ps = ps1.tile([128, NE], F32, tag="g")
                nc.tensor.matmul(q_ps[h:h + 64, :], w0abc_s[:], rT_s[:], start=True, stop=True)
                nc.scalar.copy(qT[h:h + 64, :], q_ps[h:h + 64, :])
                qs_ps = ps1.tile([128, SLAB], F32, tag="g")
                nc.tensor.matmul(qs_ps[h:h + 64, :], w0abc_s[:], rsT_s[:], start=True, stop=True)
                nc.scalar.copy(qs[h:h + 64, :], qs_ps[h:h + 64, :])

            # p slab
            slab = big.tile([128, (SLAB // 2) * NE], F32)
            tsum_u = sb.tile([128, SLAB // 2], F32)
            tsum_d = sb.tile([128, SLAB // 2], F32)
            msu = sb.tile([128, SLAB // 2], F32)
            msd = sb.tile([128, SLAB // 2], F32)

            def chunk(jl):
                h = 0 if jl < SLAB // 2 else 64
                col = jl % (SLAB // 2)
                return slab[h:h + 64, col * NE:(col + 1) * NE], h, col

            for jl in range(SLAB):
                dst, h, col = chunk(jl)
                q = jl // 16; u = jl % 16
                w_ps = ps.tile([128, NE], F32, tag="pp")
                nc.tensor.matmul(w_ps[h:h + 64, :], w0d4[32 * q:32 * q + 1, :],
                                 rrflat4[32 * q:32 * q + 1, u * NE:(u + 1) * NE],
                                 start=True, stop=True, tile_position=(32 * q, h))
                t1 = lp.tile([128, NE], F32, tag="t1")
                qcol = qs[h:h + 64, jl:jl + 1]
                _apx(qcol, [[0, NE]])
                nc.vector.tensor_tensor(out=t1[h:h + 64, :], in0=qcol, in1=qT[h:h + 64, :], op=AL.subtract)
                nc.vector.tensor_tensor(out=t1[h:h + 64, :], in0=t1[h:h + 64, :], in1=w_ps[h:h + 64, :], op=AL.add)
                nc.scalar.activation(dst[:, 0:NU], t1[h:h + 64, 0:NU], AF.Tanh, bias=b0c[h:h + 64, 0:1],
                                     accum_out=tsum_u[h:h + 64, col:col + 1])
                nc.scalar.activation(dst[:, NU:NE], t1[h:h + 64, NU:NE], AF.Tanh, bias=b0c[h:h + 64, 0:1],
                                     accum_out=tsum_d[h:h + 64, col:col + 1])
            nc.vector.tensor_copy(msu[:], tsum_u[:])
            nc.vector.tensor_copy(msd[:], tsum_d[:])

            # p0 means + AG0
            SUu = sb.tile([3, 1], F32); SUd = sb.tile([3, 1], F32)
            nc.vector.tensor_reduce(SUu[:], rT_s[:, 0:NU], axis=X, op=AL.add)
            nc.vector.tensor_reduce(SUd[:], rT_s[:, NU:NE], axis=X, op=AL.add)
            pm0u = sb.tile([3, SLAB], F32); pm0d = sb.tile([3, SLAB], F32)
            su_b = SUu[:, 0:1]; _apx(su_b, [[0, SLAB]])
            sd_b = SUd[:, 0:1]; _apx(sd_b, [[0, SLAB]])
            nc.vector.scalar_tensor_tensor(out=pm0u[:], in0=rsT_s[:], scalar=float(NU), in1=su_b,
                                           op0=AL.mult, op1=AL.subtract)
            nc.vector.scalar_tensor_tensor(out=pm0d[:], in0=rsT_s[:], scalar=float(NU), in1=sd_b,
                                           op0=AL.mult, op1=AL.subtract)
            lsum = ps1.tile([SLAB, 1], F32, tag="g")
            nc.tensor.matmul(lsum[:], rrlen_t[:, 0:SLAB], ones128[:], start=True, stop=False)
            nc.tensor.matmul(lsum[:], rrlen_t[:, SLAB:2 * SLAB], ones128[:], start=False, stop=True)
            lenu = sb.tile([SLAB, 1], F32); nc.scalar.copy(lenu[:], lsum[:])
            lsum2 = ps1.tile([SLAB, 1], F32, tag="g")
            nc.tensor.matmul(lsum2[:], rrlen_t[:, 2 * SLAB:3 * SLAB], ones128[:], start=True, stop=False)
            nc.tensor.matmul(lsum2[:], rrlen_t[:, 3 * SLAB:4 * SLAB], ones128[:], start=False, stop=True)
            lend = sb.tile([SLAB, 1], F32); nc.scalar.copy(lend[:], lsum2[:])

            ag0_in = dr.tile([8, SLAB], F32)
            ag0_out = dr.tile([64, SLAB], F32)
            nc.sync.dma_start(_dap(ag0_in[:], [[SLAB, 3], [1, SLAB]], 0), pm0u[:])
            nc.sync.dma_start(_dap(ag0_in[:], [[1, SLAB]], 3 * SLAB), lenu[:])
            nc.sync.dma_start(_dap(ag0_in[:], [[SLAB, 3], [1, SLAB]], 4 * SLAB), pm0d[:])
            nc.sync.dma_start(_dap(ag0_in[:], [[1, SLAB]], 7 * SLAB), lend[:])
            nc.gpsimd.collective_compute("AllGather", AL.bypass, replica_groups=AG8,
                                         ins=[ag0_in.opt()], outs=[ag0_out.opt()])
            pm0T = sb.tile([8, NE], F32)
            nc.sync.dma_start(pm0T[0:3, :], _dap(ag0_out[:], [[SLAB, 3], [8 * SLAB, 8], [1, SLAB]], 0))
            nc.sync.dma_start(pm0T[3:4, :], _dap(ag0_out[:], [[8 * SLAB, 8], [1, SLAB]], 3 * SLAB))
            nc.sync.dma_start(pm0T[4:7, :], _dap(ag0_out[:], [[SLAB, 3], [8 * SLAB, 8], [1, SLAB]], 4 * SLAB))
            nc.sync.dma_start(pm0T[7:8, :], _dap(ag0_out[:], [[8 * SLAB, 8], [1, SLAB]], 7 * SLAB))

            def p_layer(l):
                Wl = wp.tile([128, NPV], F32, tag="Wl")
                nc.sync.dma_start(Wl[0:64, :], D[f"Wr{l}"][:])
                nc.sync.dma_start(Wl[64:128, :], D[f"Wr{l}"][:])
                blc = wp.tile([128, 1], F32, tag="blc")
                nc.sync.dma_start(blc[0:64, :], D[f"Wrb{l}"][:])
                nc.sync.dma_start(blc[64:128, :], D[f"Wrb{l}"][:])
                for jl in range(SLAB):
                    dst, h, col = chunk(jl)
                    w_ps = ps.tile([128, NE], F32, tag="pp")
                    nc.tensor.matmul(w_ps[h:h + 64, :], Wl[h:h + 64, :], dst, start=True, stop=True)
                    tmp = lp.tile([128, NE], F32, tag="t1")
                    nc.scalar.activation(tmp[h:h + 64, 0:NU], w_ps[h:h + 64, 0:NU], AF.Tanh,
                                         bias=blc[h:h + 64, 0:1], accum_out=tsum_u[h:h + 64, col:col + 1])
                    nc.scalar.activation(tmp[h:h + 64, NU:NE], w_ps[h:h + 64, NU:NE], AF.Tanh,
                                         bias=blc[h:h + 64, 0:1], accum_out=tsum_d[h:h + 64, col:col + 1])
                    nc.vector.tensor_tensor(out=dst, in0=dst, in1=tmp[h:h + 64, :], op=AL.add)
                nc.vector.tensor_tensor(out=msu[:], in0=msu[:], in1=tsum_u[:], op=AL.add)
                nc.vector.tensor_tensor(out=msd[:], in0=msd[:], in1=tsum_d[:], op=AL.add)

            def ag_means(idx):
                gi = dr.tile([128, SLAB], F32, tag=f"agi{idx}")
                go = dr.tile([1024, SLAB], F32, tag=f"ago{idx}")
                nc.sync.dma_start(_dap(gi[:], [[SLAB, 64], [1, SLAB // 2]], 0), msu[0:64, :])
                nc.sync.dma_start(_dap(gi[:], [[SLAB, 64], [1, SLAB // 2]], SLAB // 2), msu[64:128, :])
                nc.sync.dma_start(_dap(gi[:], [[SLAB, 64], [1, SLAB // 2]], 64 * SLAB), msd[0:64, :])
                nc.sync.dma_start(_dap(gi[:], [[SLAB, 64], [1, SLAB // 2]], 64 * SLAB + SLAB // 2), msd[64:128, :])
                nc.gpsimd.collective_compute("AllGather", AL.bypass, replica_groups=AG8,
                                             ins=[gi.opt()], outs=[go.opt()])
                pm = sb.tile([128, NE], F32, tag=f"pm{idx}")
                nc.sync.dma_start(pm[0:64, :], _dap(go[:], [[SLAB, 64], [128 * SLAB, 8], [1, SLAB]], 0))
                nc.sync.dma_start(pm[64:128, :], _dap(go[:], [[SLAB, 64], [128 * SLAB, 8], [1, SLAB]], 64 * SLAB))
                return pm

            pms = [None] * 5
            pms[1] = ag_means(1)
            for l in range(3):
                p_layer(l)
                pms[l + 2] = ag_means(l + 2)

            # s_v0
            sva = sb.tile([48, NE], F32)
            sv_ps = ps1.tile([48, NE], F32, tag="acc")
            nc.tensor.matmul(sv_ps[:], sel3_s[:], rT_s[:], start=True, stop=True)
            nc.vector.tensor_scalar_sub(out=sva[:], in0=sv_ps[:], scalar1=aoff_s[:, 0:1])
            svb = sb.tile([NA, NE], F32)
            ar_ps = ps1.tile([NA, NE], F32, tag="acc")
            nc.tensor.matmul(ar_ps[:], aT3_s[:], rT_s[:], start=True, stop=True)
            n2r16_ps = ps1.tile([NA, NE], F32, tag="g")
            nc.tensor.matmul(n2r16_ps[:], ones1r[0:1, 0:NA], n2row[:], start=True, stop=True)
            n2r16 = sb.tile([NA, NE], F32)
            nc.scalar.copy(n2r16[:], n2r16_ps[:])
            nc.vector.scalar_tensor_tensor(out=svb[:], in0=ar_ps[:], scalar=-2.0, in1=n2r16[:],
                                           op0=AL.mult, op1=AL.add)
            nc.vector.tensor_scalar_add(out=svb[:], in0=svb[:], scalar1=na2_s[:, 0:1])
            nc.scalar.sqrt(svb[:], svb[:])

            e16 = sb.tile([NA, NE], F32)
            nc.scalar.activation(e16[:], svb[:], AF.Exp, scale=-1.0)
            esc_ps = ps1.tile([1, NE], F32, tag="g")
            nc.tensor.matmul(esc_ps[:], ones16[:], e16[:], start=True, stop=True)
            esc = sb.tile([1, NE], F32); nc.scalar.copy(esc[:], esc_ps[:])
            escrep_ps = ps1.tile([128, NE], F32, tag="g")
            nc.tensor.matmul(escrep_ps[:], ones1r[:], esc[:], start=True, stop=True)
            escrep = sb.tile([128, NE], F32); nc.scalar.copy(escrep[:], escrep_ps[:])

            def mean_bcast(dst, src, c0, c1, pn):
                red = lp.tile([pn, 1], F32, tag="red")
                nc.vector.tensor_reduce(red[0:pn, :], src[0:pn, c0:c1], axis=X, op=AL.add)
                rb = red[0:pn, 0:1]; _apx(rb, [[0, NE]])
                nc.vector.tensor_copy(dst, rb)

            suA = sb.tile([48, NE], F32); suB = sb.tile([NA, NE], F32)
            sdA = sb.tile([48, NE], F32); sdB = sb.tile([NA, NE], F32)
            mean_bcast(suA[:], sva, 0, NU, 48); mean_bcast(suB[:], svb, 0, NU, NA)
            mean_bcast(sdA[:], sva, NU, NE, 48); mean_bcast(sdB[:], svb, NU, NE, NA)

            sv0t = sb.tile([128, NE], F32, tag="sva0")
            sv1t = sb.tile([128, NE], F32, tag="sva1")
            sv_tiles = [sv0t, sv1t]
            su0p = sb.tile([128, NE], F32)
            su1p = sb.tile([128, NE], F32)
            sd0p = sb.tile([128, NE], F32)
            sd1p = sb.tile([128, NE], F32)
            nv0p = sb.tile([128, NE], F32)
            nv1p = sb.tile([128, NE], F32)
            sh0p = sb.tile([128, NU], F32)
            sh1p = sb.tile([128, NU], F32)
            chunks0 = [(suA, 0, 48), (suB, 48, 16), (sdA, 64, 48), (sdB, 112, 16),
                       (pm0T, 128, 8), (sva, 136, 48), (svb, 184, 16)]
            v0b0 = sb.tile([128, 1], F32, tag="v0b0")
            v0b1 = sb.tile([128, 1], F32, tag="v0b1")
            v0b_c = [v0b0, v0b1]
            nc.sync.dma_start(v0b_c[0][:], V0b[0:128, :])
            nc.sync.dma_start(v0b_c[1][:], V0b[128:256, :])
            w0tiles = []
            for ci, (ct, r0, kk) in enumerate(chunks0):
                wt = wp.tile([kk, NSV], F32, tag=f"v0w{ci}")
                nc.sync.dma_start(wt[:], V0p[r0:r0 + kk, :])
                w0tiles.append(wt)
            for m in range(2):
                acc = ps1.tile([128, NE], F32, tag="acc")
                for ci, (ct, r0, kk) in enumerate(chunks0):
                    nc.tensor.matmul(acc[:], w0tiles[ci][:, m * 128:(m + 1) * 128], ct[0:kk, :],
                                     start=(ci == 0), stop=(ci == 6))
                nc.scalar.activation(sv_tiles[m][:], acc[:], AF.Tanh, bias=v0b_c[m][:, 0:1])

            def s_blocks(pm):
                su = [su0p, su1p]; sd = [sd0p, sd1p]
                for m in range(2):
                    mean_bcast(su[m][:], sv_tiles[m], 0, NU, 128)
                    mean_bcast(sd[m][:], sv_tiles[m], NU, NE, 128)
                return [(su[0], 0), (su[1], 128), (sd[0], 256), (sd[1], 384),
                        (pm, 512), (sv_tiles[0], 640), (sv_tiles[1], 768)]

            for l in range(3):
                blocks = s_blocks(pms[l + 1])
                vb0 = lp.tile([128, 1], F32, tag="vb0")
                vb1 = lp.tile([128, 1], F32, tag="vb1")
                vb_c = [vb0, vb1]
                nc.sync.dma_start(vb_c[0][:], D[f"Vrb{l}"][0:128, :])
                nc.sync.dma_start(vb_c[1][:], D[f"Vrb{l}"][128:256, :])
                wts = []
                for ci, (ct, r0) in enumerate(blocks):
                    wt = wp.tile([128, NSV], F32, tag=f"vw{ci}")
                    nc.sync.dma_start(wt[:], D[f"Vr{l}"][r0:r0 + 128, :])
                    wts.append(wt)
                newv = [nv0p, nv1p]
                for m in range(2):
                    acc = ps1.tile([128, NE], F32, tag="acc")
                    for ci, (ct, r0) in enumerate(blocks):
                        nc.tensor.matmul(acc[:], wts[ci][:, m * 128:(m + 1) * 128], ct[:],
                                         start=(ci == 0), stop=(ci == 6))
                    nc.scalar.activation(newv[m][:], acc[:], AF.Tanh, bias=vb_c[m][:, 0:1])
                for m in range(2):
                    nc.vector.tensor_tensor(out=sv_tiles[m][:], in0=sv_tiles[m][:], in1=newv[m][:], op=AL.add)

            blocks4 = s_blocks(pms[4])

            def head(Vw, Vbn, c0, c1, wn, wbn, e0, e1, outdram):
                vb0 = lp.tile([128, 1], F32, tag="vb0")
                vb1 = lp.tile([128, 1], F32, tag="vb1")
                vb_c = [vb0, vb1]
                nc.sync.dma_start(vb_c[0][:], D[Vbn][0:128, :])
                nc.sync.dma_start(vb_c[1][:], D[Vbn][128:256, :])
                wts = []
                for ci, (ct, r0) in enumerate(blocks4):
                    wt = wp.tile([128, NSV], F32, tag=f"vw{ci}")
                    nc.sync.dma_start(wt[:], D[Vw][r0:r0 + 128, :])
                    wts.append(wt)
                sh = [sh0p, sh1p]
                for m in range(2):
                    acc = ps1.tile([128, NU], F32, tag="acc")
                    for ci, (ct, r0) in enumerate(blocks4):
                        nc.tensor.matmul(acc[:], wts[ci][:, m * 128:(m + 1) * 128], ct[:, c0:c1],
                                         start=(ci == 0), stop=(ci == 6))
                    nc.scalar.activation(sh[m][:], acc[:], AF.Tanh, bias=vb_c[m][:, 0:1])
                wb0 = lp.tile([128, 1], F32, tag="wb0")
                wb1 = lp.tile([128, 1], F32, tag="wb1")
                wb_c = [wb0, wb1]
                nc.sync.dma_start(wb_c[0][:], D[wbn][0:128, :])
                nc.sync.dma_start(wb_c[1][:], D[wbn][128:256, :])
                for m in range(2):
                    acc = ps1.tile([128, NU], F32, tag="acc")
                    for kk in range(2):
                        wt = wp.tile([128, 128], F32, tag=f"ow{kk}")
                        nc.sync.dma_start(wt[:], D[wn][kk * 128:(kk + 1) * 128, m * 128:(m + 1) * 128])
                        nc.tensor.matmul(acc[:], wt[:], sh[kk][:], start=(kk == 0), stop=(kk == 1))
                    ot = lp.tile([128, NU], F32, tag="ot")
                    nc.vector.scalar_tensor_tensor(out=ot[:], in0=acc[:], scalar=wb_c[m][:, 0:1],
                                                   in1=escrep[:, e0:e1], op0=AL.add, op1=AL.mult)
                    nc.sync.dma_start(outdram[m * 128:(m + 1) * 128, :], ot[:])

            head("Vhu", "Vhub", 0, NU, "wu", "wub", 0, NU, orbu)
            head("Vhd", "Vhdb", NU, NE, "wd", "wdb", NU, NE, orbd)

    nc.compile()
    return nc


_NC = None


def _prep(inputs):
    r = np.asarray(inputs["r"], np.float32)
    a = np.asarray(inputs["a"], np.float32)
    perm = np.zeros(64, dtype=np.int64)
    for k in range(4):
        for at in range(16):
            perm[16 * k + at] = 4 * at + k
    V0 = np.asarray(inputs["V0_W"], np.float32)
    V0p = np.zeros_like(V0)
    V0p[0:64] = V0[0:64][perm] / NU
    V0p[64:128] = V0[64:128][perm] / NU
    V0p[128:136] = V0[128:136] / NU
    V0p[136:200] = V0[136:200][perm]

    def scale_V(V):
        Vs = np.asarray(V, np.float32).copy()
        Vs[0:512] /= NU
        Vs[512:640] /= NU
        return Vs

    aoff = np.zeros((48, 1), np.float32)
    for k in range(3):
        aoff[16 * k:16 * k + 16, 0] = a[:, k]
    sel3 = np.zeros((3, 48), np.float32)
    for k in range(3):
        sel3[k, 16 * k:16 * k + 16] = 1.0
    base = dict(
        rT=np.ascontiguousarray(r.T), aT3=np.ascontiguousarray(a.T),
        aoff=aoff, na2=(a * a).sum(1).reshape(16, 1).astype(np.float32),
        V0p=V0p, V0b=np.asarray(inputs["V0_b"], np.float32).reshape(-1, 1),
        W0abc=np.ascontiguousarray(np.asarray(inputs["W0_W"], np.float32)[0:3]),
        W0d=np.ascontiguousarray(np.asarray(inputs["W0_W"], np.float32)[3:4]),
        W0b=np.asarray(inputs["W0_b"], np.float32).reshape(-1, 1),
        Vhu=scale_V(inputs["Vhu_W"]), Vhub=np.asarray(inputs["Vhu_b"], np.float32).reshape(-1, 1),
        Vhd=scale_V(inputs["Vhd_W"]), Vhdb=np.asarray(inputs["Vhd_b"], np.float32).reshape(-1, 1),
        wu=np.asarray(inputs["wu_W"], np.float32), wub=np.asarray(inputs["wu_b"], np.float32).reshape(-1, 1),
        wd=np.asarray(inputs["wd_W"], np.float32), wdb=np.asarray(inputs["wd_b"], np.float32).reshape(-1, 1),
        sel3=sel3,
    )
    for l in range(3):
        base[f"Vr{l}"] = scale_V(np.asarray(inputs["Vr_W"], np.float32)[l])
        base[f"Vrb{l}"] = np.asarray(inputs["Vr_b"], np.float32)[l].reshape(-1, 1)
        base[f"Wr{l}"] = np.asarray(inputs["Wr_W"], np.float32)[l]
        base[f"Wrb{l}"] = np.asarray(inputs["Wr_b"], np.float32)[l].reshape(-1, 1)
    in_maps = []
    for c in range(8):
        m = dict(base)
        m["rsT"] = np.ascontiguousarray(r[64 * c:64 * c + 64].T)
        d3 = np.zeros((4, 128, SLAB), np.float32)
        for jl in range(SLAB):
            i = 64 * c + jl
            d3[i // 128, i % 128, jl] = 3.0
        m["diag3"] = d3
        in_maps.append(m)
    return in_maps


def kernel(**inputs):
    global _NC
    if _NC is None:
        _NC = build()
    in_maps = _prep(inputs)
    res = run_bass_kernel_spmd(_NC, in_maps, core_ids=list(range(8)))
    ou = np.asarray(res.results[0]["orbu"], np.float32)
    od = np.asarray(res.results[0]["orbd"], np.float32)
    sgu, lu = np.linalg.slogdet(ou)
    sgd, ld = np.linalg.slogdet(od)
    with np.errstate(over="ignore", under="ignore", divide="ignore"):
        psi = np.float32(sgu * sgd) * np.exp(np.float32(lu + ld))
        out = np.log(np.abs(psi))
    return np.float32(out)
